# revision 7
# baseline (speedup 1.0000x reference)
"""Trainium2 Bass kernel for nn_Decoder (gnn_message_passing) — v2.

Math (per batch b, agent a):
    s[b,a]  = abs_actions[b, idx[b,a]]                     (host gather)
    z[b,a,:] = s[b,a] * W1[0,:] + e[a,:],  e = emb @ W1[1:] + b1   (host e)
    out[b,a,:] = relu(z) @ W2 + b2

Device algorithm (per core, hT layout z[h, a], data-parallel over B):
  - 3 rotating PSUM z-slots [128, 1024] (two h-chunks side by side),
    seeded with e via fp8-DoubleRow identity matmuls (e shipped as an
    fp8 hi + 16x-scaled-lo pair; residual ~3e-4 of e).  Each slot is
    re-seeded every RESEED-th use to bound the fp8 Δ-residual random
    walk; on those batches the rhs carries s_b instead of Δs.
  - Per batch one fp8-DoubleRow transition matmul per h-chunk (N=512,
    0.5 cyc/row, ~107 ns) adds w ⊗ (s_b - s_{b-3}): rhs rows are the
    fp8 hi/lo split of Δs (and Δs/16 for the w_lo partner rows), lhsT
    rows are [w_hi, w_hi, 16*w_lo, 16*w_lo].  The rotation telescopes
    so a slot always holds z_b = e + w ⊗ s_b.
  - relu evacuation PSUM->SBUF fp16 alternates DVE/ACT strictly; the
    steady state is evac-engine-bound at ~596 ns/batch (ACT 1038 ns,
    DVE 1192 ns per [128, 1024] slot in the cost model).
  - Stage 2 is inverted: h-tile slices are the STATIONARY operand and
    W2 chunks [128, 2] (fp16) the moving one, so each of the 8 matmuls
    per batch streams only 2 rows (~1 ns).  Outputs accumulate into a
    per-block o2 PSUM bank at column 128*j + 2*g (bank zeroed once per
    block by a start=True zeros-matmul — a start=True reset is
    bank-granular, so per-batch start flags would corrupt neighbors);
    evacuated in two chunks per block (g==55 partial, g==63 remainder)
    to shorten the tail.
"""

import numpy as np
import ml_dtypes

import concourse.bass as bass
import concourse.bacc as bacc
import concourse.mybir as mybir
import concourse.tile as tile
from concourse import bass_utils

F32 = mybir.dt.float32
BF16 = mybir.dt.bfloat16
FP16 = mybir.dt.float16
FP8 = mybir.dt.float8e4

B, A, NABS, E, H, OUT = 2048, 512, 16, 256, 256, 2
NCORES = 8
BC = B // NCORES  # batches per core
NE = 3  # rotating z slots
RESEED = 16  # reseed a slot every RESEED-th use (bounds fp8 error walk)
GB = 32  # Δ DMA group size (batches)

AF = mybir.ActivationFunctionType
PM = mybir.MatmulPerfMode

FP8NP = ml_dtypes.float8_e4m3
BF16NP = ml_dtypes.bfloat16
FP16NP = np.float16


def _build(nb: int):
    assert nb % 64 == 0
    ngroups = nb // GB
    nc = bacc.Bacc(
        "TRN2", target_bir_lowering=False, debug=False, num_devices=NCORES
    )

    eidr_d = nc.dram_tensor("eidr", [128, 2, 1152], FP8, kind="ExternalInput").ap()
    wdr_d = nc.dram_tensor("wdr", [2, 2, 256], FP8, kind="ExternalInput").ap()
    w2_d = nc.dram_tensor("w2", [128, 4], FP16, kind="ExternalInput").ap()
    dlt_d = nc.dram_tensor(
        "dlt", [ngroups, 2, 2, GB * 512], FP8, kind="ExternalInput"
    ).ap()
    out_d = nc.dram_tensor(
        "out", [nb // 64, 128, 512], F32, kind="ExternalOutput"
    ).ap()

    with tile.TileContext(nc) as tc:
        with (
            tc.tile_pool(name="const", bufs=1) as cpool,
            tc.tile_pool(name="dpool", bufs=3) as dpool,
            tc.tile_pool(name="h", bufs=6) as hpool,
            tc.tile_pool(name="osb", bufs=4) as opool,
            tc.tile_pool(name="zps", bufs=NE, space="PSUM") as zpool,
            tc.tile_pool(name="o2", bufs=2, space="PSUM") as o2pool,
        ):
            # ---- constants (queue order = priority order) ----
            eidr = cpool.tile([128, 2, 1152], FP8, tag="eidr")
            nc.sync.dma_start(eidr[:], eidr_d)
            e_sb = eidr[:, :, 0:1024]
            idr = eidr[:, :, 1024:1152]
            wdr = cpool.tile([2, 2, 256], FP8, tag="wdr")
            nc.sync.dma_start(wdr[:], wdr_d)

            def fetch_group(gg):
                t = dpool.tile([2, 2, GB * 512], FP8, tag="dlt")
                nc.sync.dma_start(t[:], dlt_d[gg])
                return t

            dtiles = {0: fetch_group(0)}
            w2sb = cpool.tile([128, 4], FP16, tag="w2sb")
            nc.sync.dma_start(w2sb[:], w2_d)
            zk = cpool.tile([1, 512], BF16, tag="zk")
            nc.gpsimd.memset(zk[:], 0.0)
            if ngroups > 1:
                dtiles[1] = fetch_group(1)

            etiles = [
                zpool.tile([128, 1024], F32, tag="z", name=f"z{t}")
                for t in range(NE)
            ]

            # ---- batch loop ----
            o2 = None
            for b in range(nb):
                gg = b // GB
                if b % GB == 0 and gg + 2 < ngroups:
                    dtiles[gg + 2] = fetch_group(gg + 2)
                if gg - 1 in dtiles:
                    del dtiles[gg - 1]
                et = etiles[b % NE]
                dlt = dtiles[gg]
                doff = (b % GB) * 512
                g = b % 64
                if g == 0:
                    o2 = o2pool.tile([128, 512], F32, tag="o2")
                    # zero the whole bank once; all stage-2 matmuls accumulate
                    nc.tensor.matmul(
                        o2[:, 0:512], zk[:, 0:128], zk[:, 0:512],
                        start=True, stop=False, skip_group_check=True,
                    )

                if (b // NE) % RESEED == 0:
                    # periodic reseed: z = e (fp8 hi + scaled-lo inject)
                    for c in range(2):
                        nc.tensor.matmul(
                            et[:, c * 512 : (c + 1) * 512],
                            idr[:],
                            e_sb[:, :, c * 512 : (c + 1) * 512],
                            start=True,
                            stop=False,
                            perf_mode=PM.DoubleRow,
                            skip_group_check=True,
                        )

                # transition: z += w ⊗ Δs  (fp8 DoubleRow, 2 h-chunks)
                for c in range(2):
                    nc.tensor.matmul(
                        et[:, c * 512 : (c + 1) * 512],
                        wdr[:, :, c * 128 : (c + 1) * 128],
                        dlt[:, :, doff : doff + 512],
                        start=False,
                        stop=True,
                        perf_mode=PM.DoubleRow,
                        skip_group_check=True,
                    )

                # relu evacuation PSUM -> SBUF fp16 (strict A/D alternation)
                ht = hpool.tile([128, 1024], FP16, tag="h")
                if b % 2 == 1:
                    nc.vector.tensor_scalar_max(ht[:], et[:], 0.0)
                else:
                    nc.scalar.activation(ht[:], et[:], AF.Relu)

                # stage 2: o2[a, 128j+2g : +2] += h_b.T @ W2 (2-row moving)
                for c in range(2):
                    for j in range(4):
                        nc.tensor.matmul(
                            o2[:, 128 * j + 2 * g : 128 * j + 2 * g + 2],
                            ht[:, c * 512 + 128 * j : c * 512 + 128 * j + 128],
                            w2sb[:, 2 * c : 2 * c + 2],
                            start=False,
                            stop=(c == 1),
                            skip_group_check=True,
                        )

                # block evacuation: partial at g==55 (cols of g<56), rest at 63
                if g == 55:
                    blk = b // 64
                    o56 = opool.tile([128, 4, 112], F32, tag="o56")
                    nc.scalar.copy(
                        o56[:],
                        o2[:, 0:512].rearrange("p (j c) -> p j c", j=4)[:, :, 0:112],
                    )
                    nc.sync.dma_start(
                        out_d[blk].rearrange("p (j c) -> p j c", j=4)[:, :, 0:112],
                        o56[:],
                    )
                if g == 63:
                    blk = b // 64
                    o8 = opool.tile([128, 4, 16], F32, tag="o8")
                    nc.scalar.copy(
                        o8[:],
                        o2[:, 0:512].rearrange("p (j c) -> p j c", j=4)[:, :, 112:128],
                    )
                    nc.sync.dma_start(
                        out_d[blk].rearrange("p (j c) -> p j c", j=4)[:, :, 112:128],
                        o8[:],
                    )

    nc.finalize()
    return nc


_CACHE = {}


def _get_module(nb: int):
    if nb not in _CACHE:
        _CACHE[nb] = _build(nb)
    return _CACHE[nb]


def _fp8_hilo(x):
    hi = x.astype(FP8NP)
    lo = (x - hi.astype(np.float32)).astype(FP8NP)
    return hi, lo


def _prep_host(state, abs_actions, assignments, embed_table, W1, b1, W2, b2, nb):
    idx = np.asarray(assignments).astype(np.int64)
    absf = np.asarray(abs_actions, dtype=np.float32)
    W1 = np.asarray(W1, dtype=np.float32)
    W2 = np.asarray(W2, dtype=np.float32)
    b1 = np.asarray(b1, dtype=np.float32)
    b2 = np.asarray(b2, dtype=np.float32)
    emb = np.asarray(embed_table, dtype=np.float32)

    # shared constants
    w = W1[0]  # [H]
    w_hi = w.astype(FP8NP)
    w_lo16 = ((w - w_hi.astype(np.float32)) * 16.0).astype(FP8NP)
    wdr = np.zeros((2, 2, 256), FP8NP)
    wdr[0, 0] = w_hi
    wdr[0, 1] = w_hi
    wdr[1, 0] = w_lo16
    wdr[1, 1] = w_lo16

    e = emb @ W1[1:] + b1  # [A, H] f32
    e_hT = e.T.copy()  # [H, A]
    e_flat = np.concatenate([e_hT[0:128], e_hT[128:256]], axis=1)  # [128, 1024]
    e_hi = e_flat.astype(FP8NP)
    e_lo16 = ((e_flat - e_hi.astype(np.float32)) * 16.0).astype(FP8NP)
    e_dr = np.stack([e_hi, e_lo16], axis=1)  # [128, 2, 1024]
    idr = np.zeros((128, 2, 128), FP8NP)
    idr[:, 0, :] = np.eye(128, dtype=np.float32).astype(FP8NP)
    idr[:, 1, :] = (np.eye(128, dtype=np.float32) / 16.0).astype(FP8NP)
    eidr = np.concatenate([e_dr, idr], axis=2)  # [128, 2, 1152]

    w2sb = np.zeros((128, 4), FP16NP)
    w2sb[:, 0:2] = W2[0:128].astype(FP16NP)
    w2sb[:, 2:4] = W2[128:256].astype(FP16NP)

    s_full = np.take_along_axis(absf, idx, axis=1)  # [B, A] f32

    ngroups = nb // GB
    in_maps = []
    for m in range(NCORES):
        sv = s_full[m * BC : m * BC + nb]  # [nb, A]
        delta = sv.copy()
        for b in range(NE, nb):
            if (b // NE) % RESEED != 0:
                delta[b] = sv[b] - sv[b - NE]
        dhi, dlo = _fp8_hilo(delta)
        dhi16, dlo16 = _fp8_hilo(delta / 16.0)
        dlt = np.zeros((ngroups, 2, 2, GB * 512), FP8NP)
        for gg in range(ngroups):
            rows = slice(gg * GB, (gg + 1) * GB)
            dlt[gg, 0, 0] = dhi[rows].reshape(-1)
            dlt[gg, 0, 1] = dlo[rows].reshape(-1)
            dlt[gg, 1, 0] = dhi16[rows].reshape(-1)
            dlt[gg, 1, 1] = dlo16[rows].reshape(-1)
        in_maps.append(
            {
                "eidr": eidr,
                "wdr": wdr,
                "w2": w2sb,
                "dlt": dlt,
            }
        )
    return in_maps, b2


def _run(nc, in_maps, tries=3):
    last = None
    for _ in range(tries):
        try:
            return bass_utils.run_bass_kernel_spmd(
                nc, in_maps, core_ids=list(range(NCORES))
            )
        except Exception as e:  # fake_nrt flakiness
            last = e
    raise last


def kernel(
    state,
    abs_actions,
    abstract_agent_assignments,
    embed_table,
    W1,
    b1,
    W2,
    b2,
    _nb: int = BC,
):
    nb = _nb
    nc = _get_module(nb)
    in_maps, b2v = _prep_host(
        state, abs_actions, abstract_agent_assignments,
        embed_table, W1, b1, W2, b2, nb,
    )
    res = _run(nc, in_maps)
    full = np.zeros((B, A, OUT), np.float32)
    for m in range(NCORES):
        scr = res.results[m]["out"]  # [nb//64, 128, 512]
        v = scr.reshape(nb // 64, 128, 4, 64, OUT)  # [blk, p, j, g, o]
        v = v.transpose(0, 3, 2, 1, 4)  # [blk, g, j, p, o]
        full[m * BC : m * BC + nb] = v.reshape(nb, A, OUT)
    full += b2v.reshape(1, 1, OUT)
    return full
